# revision 23
# baseline (speedup 1.0000x reference)
"""Trainium2 Bass kernel for nn_MixtralBinaryDiff (SwiGLU MLP with BitDelta
binary-diff weights), tensor-parallel over 8 NeuronCores.

Math (per reference):
    Wk = mean_wk + ck * sign(wk - mean_wk),  ck = mean|wk - mean_wk|  (global)
    gate = x @ W1^T ; up = x @ W3^T ; h = silu(gate) * up ; out = h @ W2^T

Sharding (Megatron-style on the intermediate dim):
    core r holds rows [r*1792,(r+1)*1792) of w1/w3 (+bases) and the matching
    columns of w2; hidden_states is replicated. Each core computes a full
    [T, HID] partial of the down-projection; a chunked bf16 ReduceScatter
    sums the partials and leaves each core with an interleaved token shard,
    which the host reassembles.

Key structure:
  - gate/up matmuls are x-stationary: the transposed x tile [128h,128t] is
    the PE stationary operand and the resident weight [128h, iloc] streams
    as the moving operand in 448-wide chunks. Loops run q-outer/k-inner so
    consecutive matmuls never share a stationary operand (no redundant
    LDWEIGHTS back-to-back; the reorder window hides each load).
  - weights live in 8 k-group SBUF tiles so the post-AllReduce coefficient
    fold unblocks the first matmuls group by group; wres3 cycles wres1's
    tags (Tile's WAR deps keep it correct).
  - bulk streaming (w/mw pass-1 loads, x f32->bf16 conversion, base temps,
    final out cast) rides the SWDGE/gpsimd path; the HWDGE path is reserved
    for latency-critical tile loads so they don't queue behind bulk.
  - rep 0 PE-transposes w1's base and sign while the PE is otherwise idle
    (sign lands pre-transposed in DRAM, base directly in wres1); later reps
    use the DMA-temp path for w1 so the rep boundary has no PE bubble.
  - xbf/gd/hd are per-t-tile DRAM tiles to avoid whole-tile WAR
    serialization between writes and reads.
  - gate evac applies SiLU directly (Act.Silu) so the up pass is a single
    multiply; the down-projection partial goes out in bf16 and the
    ReduceScatter runs in bf16, with a final SWDGE cast to f32.
"""

import os

import numpy as np

B, S, HID, INTER = 2, 2048, 4096, 14336
NCORES = 8
T = B * S
RS_MCH = 4                 # token tiles per ReduceScatter chunk

# SWDGE DRAM->DRAM x-cast verified good on HW; ACT-issued HWDGE DMAs
# verified BROKEN on this stack (garbage data) — keep everything on the
# SP ring.
SWDGE_X = os.environ.get("K_SWDGE_X", "1") == "1"   # x cast DRAM->DRAM
ACT_RING = os.environ.get("K_ACT_RING", "0") == "1"  # ACT-issued HWDGE


def build_mlp_nc(ncores, t, hid, inter, tb=256, fake_cc=False, repeat=1):
    """Build the Bass module for one core (SPMD: all cores run the same
    program on different shards). Returns (nc, input_names, out_name)."""
    import concourse.mybir as mybir
    from concourse import bass_isa
    from concourse.bass import _add_dep_helper
    import concourse.tile as tile
    from concourse import bacc

    f32 = mybir.dt.float32
    bf16 = mybir.dt.bfloat16
    Alu = mybir.AluOpType
    Act = mybir.ActivationFunctionType
    Ax = mybir.AxisListType

    iloc = inter // ncores
    KH = hid // 128            # hid k-tiles (32)
    KI = iloc // 128           # local-inter k-tiles (14)
    NT = t // 128              # token tiles (32)
    KGN = 4                    # k-tiles per weight group
    NG = KH // KGN             # weight k-groups (8)
    NQ = 4                     # moving chunks per k in gate/up
    QW = iloc // NQ            # chunk width (448)
    MT = t // 128              # token tiles in down (32)
    MCH = min(RS_MCH, MT)      # m-tiles per ReduceScatter chunk (4)
    NCHUNK = MT // MCH         # RS chunks (8)
    CH_T = MCH * 128           # tokens per chunk (512)
    RS_T = CH_T // ncores      # output rows per chunk per core (64)
    NH = 2                     # psum halves in down
    NHW = hid // NH            # free width per half (2048)
    NTOT = float(inter) * float(hid)  # coeff divisor (global count)
    FC1 = 1024                 # pass-1 column chunk for w1/w3 (fdim=hid)
    FC2 = 896                  # pass-1 column chunk for w2 (fdim=iloc)
    FCM = max(FC1, FC2)
    rg = [list(range(ncores))]

    assert hid % 128 == 0 and iloc % 128 == 0 and iloc % NQ == 0
    assert MT % MCH == 0 and CH_T % ncores == 0 and KH % KGN == 0

    nc = bacc.Bacc(None, target_bir_lowering=False, debug=False,
                   num_devices=ncores)
    hw2 = nc.scalar if ACT_RING else nc.sync

    x_ext = nc.dram_tensor("hidden_states", [t, hid], f32, kind="ExternalInput")
    w1_ext = nc.dram_tensor("w1", [iloc, hid], f32, kind="ExternalInput")
    mw1_ext = nc.dram_tensor("mean_w1", [iloc, hid], f32, kind="ExternalInput")
    w3_ext = nc.dram_tensor("w3", [iloc, hid], f32, kind="ExternalInput")
    mw3_ext = nc.dram_tensor("mean_w3", [iloc, hid], f32, kind="ExternalInput")
    w2_ext = nc.dram_tensor("w2", [hid, iloc], f32, kind="ExternalInput")
    mw2_ext = nc.dram_tensor("mean_w2", [hid, iloc], f32, kind="ExternalInput")
    # bf16 output: the ReduceScatter already sums in bf16, so the host-side
    # f32 upcast is exact and the device skips a cast pass entirely.
    out_ext = nc.dram_tensor("out", [t // ncores, hid], bf16,
                             kind="ExternalOutput")

    def fold_into(stpool, wres_ap, st_ap, cbr, width):
        # wres = st * c + wres on DVE (TensorScalarPtr is DVE-only on this
        # ISA — Pool rejects it, and a two-step ACT/DVE split measured
        # slower in the timeline sim).
        nc.vector.scalar_tensor_tensor(wres_ap, st_ap, cbr[:], wres_ap,
                                       op0=Alu.mult, op1=Alu.add)

    def load_wres_from_temps(stpool, wres_g, t_ds, s_ds, cbr, fchunk, paces,
                             dma_eng=None):
        """Build folded weight tiles from the [rows, cols] DRAM temps via
        transposed reads; one (base-load, sign-load, fold) triple per
        k-tile, pacing DMAs into the tail of the previous compute pass.
        dma_eng picks the HWDGE ring (SP by default; ACT for rep-boundary
        loads so they don't queue behind dependency-stalled writes)."""
        eng = dma_eng if dma_eng is not None else nc.sync
        for g in range(NG):
            for kk in range(KGN):
                k = g * KGN + kk
                pa = paces[k] if paces is not None else None
                j, off = (k * 128) // fchunk, (k * 128) % fchunk
                d = eng.dma_start(wres_g[g][:, kk, :],
                                  t_ds[j][:, off:off + 128],
                                  transpose=True)
                if pa is not None:
                    _add_dep_helper(d.ins, pa.ins, sync=False,
                                    reason="wres prefetch pacing")
                st = stpool.tile([128, iloc], bf16, tag="st")
                d = eng.dma_start(st[:], s_ds[j][:, off:off + 128],
                                  transpose=True)
                if pa is not None:
                    _add_dep_helper(d.ins, pa.ins, sync=False,
                                    reason="wres prefetch pacing")
                fold_into(stpool, wres_g[g][:, kk, :], st[:], cbr,
                          iloc)

    def emit_once(tc, dram, cpool, p1, p1red, rep):
        pe_path = (rep == 0)   # PE-transpose w1 temps only when PE is idle

        # ---- internal DRAM buffers ----
        xbfs = [dram.tile([128, hid], bf16, name=f"xbf{rep}_{m}")
                for m in range(NT)]
        gds = [dram.tile([128, iloc], bf16, name=f"gd{rep}_{m}")
               for m in range(NT)]
        hds = [dram.tile([128, iloc], bf16, name=f"hd{rep}_{m}")
               for m in range(NT)]
        if pe_path:
            # sign of w1, already transposed: 4 tiles of 8 k-tiles each
            s1Ts = [dram.tile([128, 2 * KGN, iloc], bf16,
                              name=f"s1T{rep}_{jj}") for jj in range(4)]
            s1ds = t1ds = None
        else:
            s1ds = [dram.tile([iloc, FC1], bf16, name=f"s1d{rep}_{j}")
                    for j in range(hid // FC1)]
            t1ds = [dram.tile([iloc, FC1], bf16, name=f"t1d{rep}_{j}")
                    for j in range(hid // FC1)]
        s3ds = [dram.tile([iloc, FC1], bf16, name=f"s3d{rep}_{j}")
                for j in range(hid // FC1)]
        t3ds = [dram.tile([iloc, FC1], bf16, name=f"t3d{rep}_{j}")
                for j in range(hid // FC1)]
        s2ds = [dram.tile([hid, FC2], bf16, name=f"s2d{rep}_{j}")
                for j in range(iloc // FC2)]
        t2ds = [dram.tile([hid, FC2], bf16, name=f"t2d{rep}_{j}")
                for j in range(iloc // FC2)]
        pout = dram.tile([t, hid], bf16, name=f"pout{rep}")
        shared = "Shared" if ncores > 4 else "Local"
        cins = [dram.tile([1, 8], f32, name=f"cin{rep}_{i}") for i in range(3)]
        couts = [dram.tile([1, 8], f32, name=f"cout{rep}_{i}",
                           addr_space=shared) for i in range(3)]
        rsout = dram.tile([t // ncores, hid], bf16, name=f"rsout{rep}")

        cbrs = [cpool.tile([128, 1], f32, name=f"cbr{rep}_{i}", tag=f"cbr{i}")
                for i in range(3)]

        # ---- pass-1 over one weight pair ----
        # Streams [128, fchunk] blocks on the SWDGE path; the sign goes out
        # via s_sink (PE transpose, rep 0 w1) or as bf16 [rows, cols] DRAM
        # temps; the base via t_sink or SWDGE-cast temps. Accumulates the
        # |d| reduction and AllReduces the scalar coefficient.
        def pass1(w_e, mw_e, s_ds, t_ds, rows, fdim, fchunk, idx,
                  t_sink=None, s_sink=None, pace_after=None):
            nt_ = rows // 128
            ncf = fdim // fchunk
            red = p1red.tile([128, nt_ * ncf], f32, name=f"red{rep}_{idx}",
                             tag=f"red{idx}")
            n = -1
            for j in range(ncf):
                for i in range(nt_):
                    n += 1
                    rs = slice(i * 128, (i + 1) * 128)
                    cs = slice(j * fchunk, (j + 1) * fchunk)
                    wt = p1.tile([128, fchunk], f32, tag="p1w",
                                 padded_shape=[128, FCM])
                    d1 = nc.gpsimd.dma_start(wt[:], w_e[rs, cs])
                    mt = p1.tile([128, fchunk], f32, tag="p1m",
                                 padded_shape=[128, FCM])
                    d2 = nc.gpsimd.dma_start(mt[:], mw_e[rs, cs])
                    if pace_after is not None:
                        pl = (pace_after if isinstance(pace_after, list)
                              else [pace_after])
                        pa = pl[min(len(pl) - 1,
                                    (n * len(pl)) // (nt_ * ncf))]
                        if pa is not None:
                            _add_dep_helper(d1.ins, pa.ins, sync=False,
                                            reason="phase pacing")
                            _add_dep_helper(d2.ins, pa.ins, sync=False,
                                            reason="phase pacing")
                    db = p1.tile([128, fchunk], bf16, tag="p1d",
                                 padded_shape=[128, FCM])
                    nc.vector.tensor_tensor(db[:], wt[:], mt[:], Alu.subtract)
                    if s_sink is not None:
                        sgf = p1.tile([128, fchunk], f32, tag="p1sf",
                                      padded_shape=[128, FCM])
                        nc.scalar.activation(sgf[:], db[:], Act.Sign)
                        s_sink(i, j, fchunk, sgf)
                    else:
                        sg = p1.tile([128, fchunk], bf16, tag="p1s",
                                     padded_shape=[128, FCM])
                        nc.scalar.activation(sg[:], db[:], Act.Sign)
                        nc.sync.dma_start(s_ds[j][rs, :], sg[:])
                    if t_sink is not None:
                        t_sink(i, j, fchunk, mt)
                    else:
                        # bf16 base temp via SWDGE cast-on-store
                        nc.gpsimd.dma_start(t_ds[j][rs, :], mt[:])
                    nc.vector.tensor_reduce(
                        red[:, i * ncf + j:i * ncf + j + 1], db[:],
                        axis=Ax.X, op=Alu.add, apply_absolute_value=True)
            redt = p1red.tile([128, 1], f32, name=f"redt{rep}_{idx}",
                              tag=f"redt{idx}")
            gate_inst = nc.vector.tensor_reduce(redt[:], red[:], axis=Ax.X,
                                                op=Alu.add)
            par = p1red.tile([128, 1], f32, name=f"par{rep}_{idx}",
                             tag=f"par{idx}")
            nc.gpsimd.partition_all_reduce(par[:], redt[:], channels=128,
                                           reduce_op=bass_isa.ReduceOp.add)
            cst = cpool.tile([1, 8], f32, name=f"cst{rep}_{idx}",
                             tag=f"cst{idx}")
            nc.vector.memset(cst[:], 0.0)
            nc.vector.tensor_copy(cst[0:1, 0:1], par[0:1, 0:1])
            nc.sync.dma_start(cins[idx][:], cst[:])
            if fake_cc:
                nc.sync.dma_start(couts[idx][:], cins[idx][:])
            else:
                nc.gpsimd.collective_compute(
                    "AllReduce", Alu.add, replica_groups=rg,
                    ins=[cins[idx][:].opt()], outs=[couts[idx][:].opt()])
            cld = cpool.tile([1, 8], f32, name=f"cld{rep}_{idx}",
                             tag=f"cld{idx}")
            nc.sync.dma_start(cld[:], couts[idx][:])
            csc = cpool.tile([1, 1], f32, name=f"csc{rep}_{idx}",
                             tag=f"csc{idx}")
            nc.vector.tensor_scalar(csc[:], cld[0:1, 0:1], 1.0 / NTOT,
                                    None, Alu.mult)
            nc.gpsimd.partition_broadcast(cbrs[idx][:], csc[:])
            return gate_inst

        with (
            tc.tile_pool(name="wres", bufs=1) as wpool,
            tc.tile_pool(name="xt", bufs=3) as xtpool,
            tc.tile_pool(name="stage", bufs=2) as stpool,
            tc.tile_pool(name="evac", bufs=2) as evpool,
            tc.tile_pool(name="xconv", bufs=2) as xcpool,
        ):
            ident = cpool.tile([128, 128], f32, name=f"ident{rep}",
                               tag="ident")
            from concourse.masks import make_identity
            make_identity(nc, ident[:])

            # weight k-group tiles: [128h, KGN, iloc] bf16; tag-cycled so
            # later phases reuse the same SBUF with WAR deps handled by Tile.
            wres1 = [wpool.tile([128, KGN, iloc], bf16, tag=f"wres{g}",
                                name=f"wres1_{rep}_{g}") for g in range(NG)]

            # pass-1 w1
            if pe_path:
                with (
                    tc.tile_pool(name="psT", bufs=2, space="PSUM") as psT,
                    tc.tile_pool(name="psS", bufs=2, space="PSUM") as psS,
                    tc.tile_pool(name="sst", bufs=2) as sstpool,
                ):
                    def w1_t_sink(i, j, fchunk, mt):
                        nb = fchunk // 128          # 8 k-tiles per chunk
                        tp = psT.tile([128, nb, 128], f32, tag="tp")
                        for b in range(nb):
                            nc.tensor.transpose(tp[:, b, :],
                                                mt[:, b * 128:(b + 1) * 128],
                                                ident[:])
                        for half in range(nb // KGN):   # 2 wres groups
                            g = j * (nb // KGN) + half
                            nc.vector.tensor_copy(
                                wres1[g][:, :, i * 128:(i + 1) * 128],
                                tp[:, half * KGN:(half + 1) * KGN, :])

                    def w1_s_sink(i, j, fchunk, sgf):
                        nb = fchunk // 128
                        tp = psS.tile([128, nb, 128], f32, tag="tps")
                        for b in range(nb):
                            nc.tensor.transpose(tp[:, b, :],
                                                sgf[:, b * 128:(b + 1) * 128],
                                                ident[:])
                        sb = sstpool.tile([128, nb, 128], bf16, tag="sst")
                        nc.vector.tensor_copy(sb[:], tp[:])
                        nc.sync.dma_start(
                            s1Ts[j][:, :, i * 128:(i + 1) * 128], sb[:])

                    g1 = pass1(w1_ext, mw1_ext, None, None, iloc, hid, FC1, 0,
                               t_sink=w1_t_sink, s_sink=w1_s_sink)
            else:
                g1 = pass1(w1_ext, mw1_ext, s1ds, t1ds, iloc, hid, FC1, 0)

            # x bf16 conversion: one SWDGE DRAM->DRAM cast per t-tile,
            # or the proven HWDGE+ACT SBUF roundtrip when SWDGE_X is off.
            def convert_x(m, pace_after=None):
                if SWDGE_X:
                    d = nc.gpsimd.dma_start(xbfs[m][:],
                                            x_ext[m * 128:(m + 1) * 128, :])
                    if pace_after is not None:
                        _add_dep_helper(d.ins, pace_after.ins, sync=False,
                                        reason="phase pacing")
                    return
                rs = slice(m * 128, (m + 1) * 128)
                for j in range(hid // 1024):
                    cs = slice(j * 1024, (j + 1) * 1024)
                    xl = xcpool.tile([128, 1024], f32, tag="xl")
                    d = nc.sync.dma_start(xl[:], x_ext[rs, cs])
                    if pace_after is not None:
                        _add_dep_helper(d.ins, pace_after.ins, sync=False,
                                        reason="phase pacing")
                    xc = xcpool.tile([128, 1024], bf16, tag="xc")
                    nc.scalar.activation(xc[:], xl[:], Act.Copy)
                    nc.sync.dma_start(xbfs[m][:, cs], xc[:])

            # Tiles 0-13 convert up front: on the Pool ring they queue
            # behind this rep's w1 pass-1, which for rep>=1 streams during
            # the previous rep's down phase (where DMA has slack). The rest
            # interleave with the gate pass, which is now ~75% DMA-loaded.
            for m in range(14):
                convert_x(m)

            # fold w1: sign comes back pre-transposed (rep 0) or via
            # transposed reads of the [i, h] temps (later reps).
            if pe_path:
                for g in range(NG):
                    for kk in range(KGN):
                        jj, b0 = g // 2, (g % 2) * KGN + kk
                        st = stpool.tile([128, iloc], bf16, tag="st")
                        nc.sync.dma_start(st[:], s1Ts[jj][:, b0, :])
                        fold_into(stpool, wres1[g][:, kk, :], st[:],
                                  cbrs[0], iloc)
            else:
                load_wres_from_temps(stpool, wres1, t1ds, s1ds, cbrs[0],
                                     FC1, None, dma_eng=hw2)

            # ---- gate / up passes (x-stationary, q-outer k-inner) ----
            # Returns per-tile marks and per-quarter marks (4/tile) for
            # fine-grained DMA pacing.
            def gateup(wres_g, is_up, interleave=None):
                marks = []
                qmarks = []
                for m in range(NT):
                    if interleave is not None:
                        interleave(m, marks)
                    xt = xtpool.tile([128, KH, 128], bf16, tag="xt")
                    hw2.dma_start(xt[:], xbfs[m][:], transpose=True)
                    pg = ps_gu.tile([128, NQ, 512], f32, tag="pg")
                    mm = None
                    for q in range(NQ):
                        for g in range(NG):
                            for kk in range(KGN):
                                k = g * KGN + kk
                                mm = nc.tensor.matmul(
                                    pg[:, q, 0:QW], xt[:, k, :],
                                    wres_g[g][:, kk, q * QW:(q + 1) * QW],
                                    start=(k == 0), stop=(k == KH - 1))
                        qmarks.append(mm)
                    if not is_up:
                        sig = evpool.tile([128, NQ, QW], bf16, tag="ho")
                        nc.scalar.activation(sig[:], pg[:, :, 0:QW],
                                             Act.Sigmoid)
                        sg = evpool.tile([128, NQ, QW], bf16, tag="sg")
                        nc.vector.tensor_tensor(sg[:], pg[:, :, 0:QW],
                                                sig[:], Alu.mult)
                        nc.sync.dma_start(
                            gds[m][:].rearrange("p (q w) -> p q w", q=NQ),
                            sg[:])
                    else:
                        sgt = evpool.tile([128, NQ, QW], bf16, tag="sg")
                        hw2.dma_start(
                            sgt[:],
                            gds[m][:].rearrange("p (q w) -> p q w", q=NQ))
                        ho = evpool.tile([128, NQ, QW], bf16, tag="ho")
                        nc.vector.tensor_tensor(ho[:], pg[:, :, 0:QW],
                                                sgt[:], Alu.mult)
                        nc.sync.dma_start(
                            hds[m][:].rearrange("p (q w) -> p q w", q=NQ),
                            ho[:])
                    marks.append(mm)
                return marks, qmarks

            with tc.tile_pool(name="psGU", bufs=2, space="PSUM") as ps_gu:
                # gate pass; convert x three tiles ahead
                def gate_il(m, marks):
                    if m + 3 < NT:
                        convert_x(m + 3,
                                  pace_after=marks[-2] if len(marks) > 1
                                  else None)

                gate_marks, gate_qm = gateup(wres1, is_up=False,
                                             interleave=gate_il)

                # pass-1 w3 paced across the gate pass, ~1 chunk-pair per
                # quarter-tile so bursts never crowd out the critical loads
                g3 = pass1(w3_ext, mw3_ext, s3ds, t3ds, iloc, hid, FC1, 1,
                           pace_after=[g1] + gate_qm[0:92])

                # wres3: transposed reads of base+sign temps, fold per k;
                # paced into the gate tail. Tag cycling reuses wres1's SBUF.
                wres3 = [wpool.tile([128, KGN, iloc], bf16, tag=f"wres{g}",
                                    name=f"wres3_{rep}_{g}")
                         for g in range(NG)]
                load_wres_from_temps(stpool, wres3, t3ds, s3ds, cbrs[1],
                                     FC1, [gate_qm[min(len(gate_qm) - 5,
                                                       76 + k)]
                                           for k in range(KH)])

                up_marks, up_qm = gateup(wres3, is_up=True)

            pass1(w2_ext, mw2_ext, s2ds, t2ds, hid, iloc, FC2, 2,
                  pace_after=[g3] + up_qm[0:92])

        # ---- down projection + chunked bf16 ReduceScatter ----
        with (
            tc.tile_pool(name="w2w", bufs=1) as w2pool,
            tc.tile_pool(name="st2", bufs=2) as st2pool,
            tc.tile_pool(name="hcol", bufs=3) as hcpool,
            tc.tile_pool(name="ot", bufs=3) as otpool,
            tc.tile_pool(name="ps3", bufs=2, space="PSUM") as ps3,
        ):
            w2ws = [w2pool.tile([128, 1, hid], bf16, name=f"w2w{rep}_{k}")
                    for k in range(KI)]
            for k in range(KI):
                pa = up_qm[min(len(up_qm) - 3, 84 + 2 * k)]
                j, off = (k * 128) // FC2, (k * 128) % FC2
                d = nc.sync.dma_start(w2ws[k][:, 0, :],
                                      t2ds[j][:, off:off + 128],
                                      transpose=True)
                _add_dep_helper(d.ins, pa.ins, sync=False,
                                reason="w2w prefetch pacing")
                for h2 in range(2):
                    hs = slice(h2 * (hid // 2), (h2 + 1) * (hid // 2))
                    s2t = st2pool.tile([128, hid // 2], bf16, tag="st2")
                    d = nc.sync.dma_start(
                        s2t[:], s2ds[j][hs, off:off + 128], transpose=True)
                    _add_dep_helper(d.ins, pa.ins, sync=False,
                                    reason="w2w prefetch pacing")
                    fold_into(st2pool, w2ws[k][:, 0, hs], s2t[:],
                              cbrs[2], hid // 2)
            for c in range(NCHUNK):
                for mi in range(MCH):
                    m = c * MCH + mi
                    ms = slice(m * 128, (m + 1) * 128)
                    hcol = hcpool.tile([128, KI, 128], bf16, tag="hcol")
                    hw2.dma_start(hcol[:], hds[m][:], transpose=True)
                    for g in range(NH):
                        gs = slice(g * NHW, (g + 1) * NHW)
                        pd = ps3.tile([128, NHW], f32, tag="pd")
                        for q in range(NHW // 512):
                            for k in range(KI):
                                qs = slice(g * NHW + q * 512,
                                           g * NHW + (q + 1) * 512)
                                nc.tensor.matmul(
                                    pd[:, q * 512:(q + 1) * 512],
                                    hcol[:, k, :], w2ws[k][:, 0, qs],
                                    start=(k == 0), stop=(k == KI - 1))
                        ot = otpool.tile([128, NHW], bf16, tag="ot")
                        nc.vector.tensor_copy(ot[:], pd[:])
                        nc.sync.dma_start(pout[ms, gs], ot[:])
                cts = slice(c * CH_T, (c + 1) * CH_T)
                crs = slice(c * RS_T, (c + 1) * RS_T)
                if fake_cc:
                    nc.sync.dma_start(rsout[crs, :],
                                      pout[c * CH_T:c * CH_T + RS_T, :])
                else:
                    nc.gpsimd.collective_compute(
                        "ReduceScatter", Alu.add, replica_groups=rg,
                        ins=[pout[cts, :].opt()], outs=[rsout[crs, :].opt()])
                # plain bf16 copy on the ACT ring (keeps Pool/SWDGE clear
                # for the next rep's weight streaming)
                hw2.dma_start(out_ext[crs, :], rsout[crs, :])

    with tile.TileContext(nc) as tc:
        with (
            tc.tile_pool(name="dram", bufs=1, space="DRAM") as dram,
            tc.tile_pool(name="consts", bufs=1) as cpool,
            tc.tile_pool(name="p1", bufs=2) as p1,
            tc.tile_pool(name="p1red", bufs=1) as p1red,
        ):
            for rep in range(repeat):
                emit_once(tc, dram, cpool, p1, p1red, rep)

    nc.compile()
    in_names = ["hidden_states", "w1", "mean_w1", "w3", "mean_w3", "w2",
                "mean_w2"]
    return nc, in_names, "out"


_CACHE = {}
LAST_RESULTS = None


def _get_built(key, *args, **kwargs):
    if key not in _CACHE:
        _CACHE[key] = build_mlp_nc(*args, **kwargs)
    return _CACHE[key]


def kernel(hidden_states, w1, mean_w1, w2, mean_w2, w3, mean_w3):
    global LAST_RESULTS
    import os
    # The axon NTFF-profile hook is unavailable in this environment and the
    # trace path would crash on import; force it off.
    os.environ["BASS_NEVER_TRACE"] = "1"
    from concourse import bass_utils

    x = np.ascontiguousarray(np.asarray(hidden_states, dtype=np.float32)
                             .reshape(T, HID))
    w1 = np.asarray(w1, dtype=np.float32)
    mean_w1 = np.asarray(mean_w1, dtype=np.float32)
    w2 = np.asarray(w2, dtype=np.float32)
    mean_w2 = np.asarray(mean_w2, dtype=np.float32)
    w3 = np.asarray(w3, dtype=np.float32)
    mean_w3 = np.asarray(mean_w3, dtype=np.float32)

    nc, in_names, out_name = _get_built("full", NCORES, T, HID, INTER)

    iloc = INTER // NCORES
    in_maps = []
    for r in range(NCORES):
        rs = slice(r * iloc, (r + 1) * iloc)
        in_maps.append({
            "hidden_states": x,
            "w1": np.ascontiguousarray(w1[rs, :]),
            "mean_w1": np.ascontiguousarray(mean_w1[rs, :]),
            "w3": np.ascontiguousarray(w3[rs, :]),
            "mean_w3": np.ascontiguousarray(mean_w3[rs, :]),
            "w2": np.ascontiguousarray(w2[:, rs]),
            "mean_w2": np.ascontiguousarray(mean_w2[:, rs]),
        })

    res = bass_utils.run_bass_kernel_spmd(nc, in_maps,
                                          core_ids=list(range(NCORES)))
    LAST_RESULTS = res

    # Reassemble: chunk c of core r holds tokens interleaved by chunk/core.
    MT = T // 128
    MCH = min(RS_MCH, MT)
    NCHUNK = MT // MCH
    CH_T = MCH * 128
    RS_T = CH_T // NCORES
    full = np.empty((T, HID), dtype=np.float32)
    for r in range(NCORES):
        o = np.asarray(res.results[r][out_name])
        for c in range(NCHUNK):
            full[c * CH_T + r * RS_T: c * CH_T + (r + 1) * RS_T] = \
                o[c * RS_T:(c + 1) * RS_T]
    return full.reshape(B, S, HID)
